# revision 1
# baseline (speedup 1.0000x reference)
"""Causal GQA self-attention (B=1, T=2048, C=1024, 16 q-heads, 4 kv-groups, d=64)
on 8 Trainium2 NeuronCores.

Sharding: tensor-parallel over heads. Core c owns q-heads (2c, 2c+1) and kv-group
c//2. Each core computes x @ w_attn for its slice (transposed layout), RoPE,
causal flash-style attention for its 2 heads, and its partial y @ w_proj
(contracting only its 128 head-dims). Host sums the 8 partial outputs.

Layout strategy (per core):
  - xT [C, T] in SBUF (strided DMA); wqkv slice [C, 256] natural.
  - qkvT = wqkv.T @ x computed transposed: qT2 [128, T] (2 heads), kvT [128, T]
    (k rows 0:64, v rows 64:128).
  - RoPE applied in [d, T] layout; the pair-rotation is a PE matmul with a
    constant +-1 permutation matrix; cos/sin are inline (baked) tables.
  - Scores computed TRANSPOSED: sT[k, q] = kT.T-free matmul, so the softmax
    denominator comes from appending a ones-column to v (one extra PE row) and
    no max-subtraction is needed (|scores| <= ~7, exp is safe in fp32).
  - att @ v computed as yT[d, q] via lhsT=v_aug, rhs=pT -- no transposes of p.
  - Normalization folded in before proj via a PE outer-product broadcast.
  - proj: out[t, :] += yT_h.T @ wproj_h per head, accumulated in PSUM.
"""

import numpy as np

import concourse.bass as bass
import concourse.mybir as mybir
from concourse import bacc
import concourse.tile as tile
from concourse.bass_utils import run_bass_kernel_spmd

T = 2048
C = 1024
D = 64
QW = 1024                     # queries processed per attention window
NCH = T // 512                # 512-wide column chunks of T
F32 = mybir.dt.float32
F32R = mybir.dt.float32r
EXP = mybir.ActivationFunctionType.Exp
MUL = mybir.AluOpType.mult
ADD = mybir.AluOpType.add

# True: load xT via strided DMA (512B contiguous runs in DRAM).
# False: load x naturally and transpose 128x128 blocks on the PE.
STRIDED_XT = True

_CACHE: dict = {}


def _rope_tables():
    # Replicate reference.apply_rope's f32 pipeline exactly.
    inv = (1.0 / (np.float32(10000.0) ** (np.arange(0, D, 2, dtype=np.float32) / np.float32(D)))).astype(np.float32)
    freqs = (np.arange(T, dtype=np.float32)[:, None] * inv[None, :]).astype(np.float32)  # (T, 32)
    freqs = np.repeat(freqs, 2, axis=1)                                                  # (T, 64)
    cos = np.cos(freqs).astype(np.float32).T.copy()                                      # (64, T)
    sin = np.sin(freqs).astype(np.float32).T.copy()
    cos2 = np.ascontiguousarray(np.concatenate([cos, cos], axis=0))                      # (128, T)
    sin2 = np.ascontiguousarray(np.concatenate([sin, sin], axis=0))
    return cos2, sin2


def _const_mats():
    # perm (as lhsT): rot[2i] = -x[2i+1], rot[2i+1] = +x[2i]
    perm = np.zeros((128, 128), np.float32)
    for i in range(64):
        perm[2 * i + 1, 2 * i] = -1.0
        perm[2 * i, 2 * i + 1] = 1.0
    ident = np.eye(128, dtype=np.float32)
    shift = np.zeros((128, 128), np.float32)   # [64+i, i] = 1 down-shift; [i, 64+i] = 1 up-shift
    for i in range(64):
        shift[64 + i, i] = 1.0
        shift[i, 64 + i] = 1.0
    kq = np.arange(128)
    binmask = np.where(kq[:, None] <= kq[None, :], 0.0, -8e29).astype(np.float32)  # [k, q] additive
    return perm, ident, shift, binmask


def _build_bass(repeat: int = 1) -> bass.Bass:
    nc = bacc.Bacc(None, target_bir_lowering=False)
    xt_d = nc.dram_tensor("xt", [C, T], F32R, kind="ExternalInput")
    wqkv = nc.dram_tensor("wqkv", [C, 4 * D], F32R, kind="ExternalInput")
    wproj = nc.dram_tensor("wproj", [2 * D, C], F32R, kind="ExternalInput")
    out = nc.dram_tensor("out", [T, C], F32, kind="ExternalOutput")

    cos2_np, sin2_np = _rope_tables()
    perm_np, ident_np, shift_np, binmask_np = _const_mats()
    cos_d = nc.inline_tensor(cos2_np, name="cos2")
    sin_d = nc.inline_tensor(sin2_np, name="sin2")
    perm_d = nc.inline_tensor(perm_np, name="permm")
    ident_d = nc.inline_tensor(ident_np, name="identm")
    shift_d = nc.inline_tensor(shift_np, name="shiftm")
    mask_d = nc.inline_tensor(binmask_np, name="binmaskm")

    with tile.TileContext(nc) as tc:
        with (
            nc.allow_low_precision(reason="fp32r rounding of matmul operands"),
            tc.tile_pool(name="const", bufs=1) as const,
            tc.tile_pool(name="big", bufs=1) as big,
            tc.tile_pool(name="work", bufs=1) as work,
            tc.tile_pool(name="ps", bufs=1, space="PSUM") as psp,
        ):
            SB = 4  # shared 1-bank psum slots

            # x^T and wqkv on the SP HWDGE queue, first in line
            w_r = const.tile([128, 8, 4 * D], F32R)
            nc.sync.dma_start(out=w_r, in_=wqkv.rearrange("(c p) n -> p c n", p=128))
            xt = big.tile([128, 8, T], F32R)  # x^T as 8 c-tiles (host-transposed input)

            def load_xt():
                for half in range(2):
                    for c in range(8):
                        nc.sync.dma_start(
                            out=xt[:, c, 1024 * half : 1024 * (half + 1)],
                            in_=xt_d[128 * c : 128 * (c + 1), 1024 * half : 1024 * (half + 1)],
                        )

            # small constants on the gpsimd (SWDGE) queue so they don't delay xt
            wpc = const.tile([128, C], F32R)
            nc.gpsimd.dma_start(out=wpc, in_=wproj[:, :])
            cos_sb = const.tile([128, T], F32)
            nc.gpsimd.dma_start(out=cos_sb, in_=cos_d[:, :])
            sin_sb = const.tile([128, T], F32)
            nc.gpsimd.dma_start(out=sin_sb, in_=sin_d[:, :])
            perm_f = const.tile([128, 128], F32)
            nc.gpsimd.dma_start(out=perm_f, in_=perm_d[:, :])
            id_sb = const.tile([128, 128], F32)
            nc.gpsimd.dma_start(out=id_sb, in_=ident_d[:, :])
            sh_f = const.tile([128, 128], F32)
            nc.gpsimd.dma_start(out=sh_f, in_=shift_d[:, :])
            mk_f = const.tile([128, 128], F32)
            nc.gpsimd.dma_start(out=mk_f, in_=mask_d[:, :])
            mk_sb = const.tile([128, 128], F32R)
            nc.vector.tensor_copy(out=mk_sb, in_=mk_f)
            idr_sb = const.tile([128, 128], F32R)
            nc.vector.tensor_copy(out=idr_sb, in_=id_sb)
            perm_sb = const.tile([128, 128], F32R)
            nc.vector.tensor_copy(out=perm_sb, in_=perm_f)
            sh_sb = const.tile([128, 128], F32R)
            nc.vector.tensor_copy(out=sh_sb, in_=sh_f)
            ones_f = const.tile([128, 64], F32)
            nc.vector.memset(ones_f, 1.0)
            ones_sb = const.tile([65, 64], F32R)
            nc.vector.tensor_copy(out=ones_sb, in_=ones_f[0:65, :])

            # persistent per-core activations
            qrope_sb = big.tile([128, T], F32R)   # roped q, h0 rows 0:64, h1 rows 64:128
            q1_sb = big.tile([64, T], F32R)       # roped q of h1 shifted to partitions 0:64
            krope_sb = big.tile([64, T], F32R)
            vaug_sb = big.tile([128, 16, D + 1], F32R)  # v tiles + ones column

            # ------- stage 1, one 512-wide chunk of T at a time -------
            rp = [0]

            def stage1_chunk(nch):
                sl = slice(512 * nch, 512 * (nch + 1))
                qraw = work.tile([128, 512], F32R, tag="qraw", bufs=2, name=f"x{rp[0]}qraw{nch}")
                kvraw = work.tile([128, 512], F32R, tag="kvraw", bufs=2, name=f"x{rp[0]}kvraw{nch}")
                for m, dst in ((0, qraw), (1, kvraw)):
                    ps = psp.tile([128, 512], F32, tag="s", bufs=SB, name=f"x{rp[0]}qkv{nch}_{m}")
                    for c in range(8):
                        nc.tensor.matmul(
                            ps,
                            lhsT=w_r[:, c, 128 * m : 128 * (m + 1)],
                            rhs=xt[:, c, sl],
                            start=(c == 0),
                            stop=(c == 7),
                        )
                    nc.vector.tensor_copy(out=dst, in_=ps)
                tmp = work.tile([128, 512], F32, tag="tmp", bufs=2, name=f"x{rp[0]}tmp{nch}")
                tmpk = work.tile([64, 512], F32, tag="tmpk", bufs=2, name=f"x{rp[0]}tmpk{nch}")
                # rope q (both heads at once)
                rps = psp.tile([128, 512], F32, tag="s", bufs=SB, name=f"x{rp[0]}rot{nch}")
                nc.tensor.matmul(rps, lhsT=perm_sb, rhs=qraw, start=True, stop=True)
                nc.vector.tensor_mul(tmp, rps, sin_sb[:, sl])
                nc.vector.tensor_mul(qrope_sb[:, sl], qraw, cos_sb[:, sl])
                nc.vector.tensor_add(qrope_sb[:, sl], qrope_sb[:, sl], tmp)
                # rope k (rows 0:64 of kvraw)
                rpsk = psp.tile([64, 512], F32, tag="s", bufs=SB, name=f"x{rp[0]}rotk{nch}")
                nc.tensor.matmul(rpsk, lhsT=perm_sb[0:64, 0:64], rhs=kvraw[0:64, :], start=True, stop=True)
                nc.vector.tensor_mul(tmpk, rpsk, sin_sb[0:64, sl])
                nc.vector.tensor_mul(krope_sb[:, sl], kvraw[0:64, :], cos_sb[0:64, sl])
                nc.vector.tensor_add(krope_sb[:, sl], krope_sb[:, sl], tmpk)
                # shift roped h1 q down to partitions 0:64
                sps = psp.tile([64, 512], F32, tag="s", bufs=SB, name=f"x{rp[0]}shift{nch}")
                nc.tensor.matmul(sps, lhsT=sh_sb[64:128, 0:64], rhs=qrope_sb[64:128, sl], start=True, stop=True)
                nc.vector.tensor_copy(out=q1_sb[:, sl], in_=sps)
                # v_aug: transpose v tiles of this chunk, append ones column
                for tt in range(4 * nch, 4 * nch + 4):
                    vps = psp.tile([128, 64], F32, tag="s", bufs=SB, name=f"x{rp[0]}vtr{tt}")
                    nc.tensor.transpose(
                        vps,
                        in_=kvraw[64:128, 128 * (tt - 4 * nch) : 128 * (tt - 4 * nch + 1)].bitcast(F32),
                        identity=id_sb[64:128, 64:128],
                    )
                    nc.vector.tensor_copy(out=vaug_sb[:, tt, 0:64], in_=vps)
                    nc.vector.tensor_copy(out=vaug_sb[:, tt, 64:65], in_=ones_f[:, 0:1])

            # ------- one attention window of QW queries (both heads interleaved) -------
            NQT = T // QW
            KPW = QW // 128
            def attn_window(i8):
                yn2 = work.tile([128, QW], F32R, tag="yn2", bufs=2, name=f"x{rp[0]}yn2_{i8}")
                ktiles = KPW * i8 + KPW
                yps = {
                    h: psp.tile([65, QW], F32, tag=f"yt{h}", bufs=1, name=f"x{rp[0]}yps_{i8}_{h}")
                    for h in range(2)
                }
                last_j = {0: KPW * i8 + 3, 1: ktiles - 1}

                def epilogue_half(a2):
                    hsl = slice(512 * a2, 512 * (a2 + 1))
                    for h in range(2):
                        r_sb = work.tile([65, 512], F32R, tag="r", bufs=2, name=f"x{rp[0]}r{i8}_{h}_{a2}")
                        nc.vector.reciprocal(out=r_sb[64:65, :], in_=yps[h][64:65, hsl])
                        rbps = psp.tile([64, 512], F32, tag="s", bufs=SB, name=f"x{rp[0]}rbp{i8}_{h}_{a2}")
                        nc.tensor.matmul(
                            rbps,
                            lhsT=ones_sb[64:65, 0:64],
                            rhs=r_sb[64:65, :],
                            start=True,
                            stop=True,
                        )
                        rb_sb = work.tile([64, 512], F32, tag="rb_sb", bufs=2, name=f"x{rp[0]}rb{i8}_{h}_{a2}")
                        nc.scalar.copy(out=rb_sb, in_=rbps)
                        if h == 0:
                            nc.vector.tensor_mul(yn2[0:64, hsl], yps[h][0:64, hsl], rb_sb)
                        else:
                            yn1 = work.tile([64, 512], F32R, tag="yn1", bufs=2, name=f"x{rp[0]}yn1_{i8}_{a2}")
                            nc.vector.tensor_mul(yn1, yps[h][0:64, hsl], rb_sb)
                            nc.gpsimd.dma_start(out=yn2[64:128, hsl], in_=yn1)
                    for t4 in range(4 * a2, 4 * a2 + 4):
                        tglob = i8 * (QW // 128) + t4
                        osb = work.tile([128, C], F32, tag="o", bufs=3, name=f"x{rp[0]}o{i8}_{t4}")
                        for n2 in range(2):
                            ops_ = psp.tile([128, 512], F32, tag="s", bufs=SB, name=f"x{rp[0]}op{i8}_{t4}_{n2}")
                            nc.tensor.matmul(
                                ops_,
                                lhsT=yn2[:, 128 * t4 : 128 * (t4 + 1)],
                                rhs=wpc[:, 512 * n2 : 512 * (n2 + 1)],
                                start=True,
                                stop=True,
                            )
                            if n2 == 0:
                                nc.vector.tensor_copy(out=osb[:, 0:512], in_=ops_)
                            else:
                                nc.scalar.copy(out=osb[:, 512:1024], in_=ops_)
                        nc.sync.dma_start(out=out[128 * tglob : 128 * (tglob + 1), :], in_=osb)

                prev = None
                for j in range(ktiles):
                    g = j - KPW * i8
                    q0 = max(g, 0) * 128
                    pts = {}
                    for h in range(2):
                        qsrc = qrope_sb if h == 0 else q1_sb
                        pt = work.tile([128, QW], F32R, tag="pt", bufs=5, name=f"x{rp[0]}pt{i8}_{j}_{h}")
                        for a2 in range(q0 // 512, 2):
                            lo = max(q0, 512 * a2)
                            hi = 512 * (a2 + 1)
                            spsm = psp.tile([128, 512], F32, tag="s", bufs=SB, name=f"x{rp[0]}s{i8}_{j}_{h}_{a2}")
                            nc.tensor.matmul(
                                spsm[:, 0 : hi - lo],
                                lhsT=krope_sb[:, 128 * j : 128 * (j + 1)],
                                rhs=qsrc[0:64, QW * i8 + lo : QW * i8 + hi],
                                start=True,
                                stop=not (lo <= q0 < hi and g >= 0),
                                skip_group_check=True,
                            )
                            if g >= 0 and lo <= q0 < hi:
                                nc.tensor.matmul(
                                    spsm[:, q0 - lo : q0 - lo + 128],
                                    lhsT=idr_sb,
                                    rhs=mk_sb,
                                    start=False,
                                    stop=True,
                                    skip_group_check=True,
                                )
                            nc.scalar.activation(
                                out=pt[:, lo:hi], in_=spsm[:, 0 : hi - lo], func=EXP, scale=0.125
                            )
                        pts[h] = pt

                    def emit_yt(jj, ptsj):
                        gg = jj - KPW * i8
                        qq0 = max(gg, 0) * 128
                        for h in range(2):
                            for a2 in range(qq0 // 512, 2):
                                lo = max(qq0, 512 * a2)
                                hi = 512 * (a2 + 1)
                                nc.tensor.matmul(
                                    yps[h][:, lo:hi],
                                    lhsT=vaug_sb[:, jj, :],
                                    rhs=ptsj[h][:, lo:hi],
                                    start=(jj == 0),
                                    stop=(jj == last_j[a2]),
                                    skip_group_check=True,
                                )

                    if prev is not None:
                        emit_yt(*prev)
                    prev = (j, pts)
                emit_yt(*prev)
                epilogue_half(0)
                epilogue_half(1)

            for _rep in range(repeat):
                rp[0] = _rep
                load_xt()
                stage1_chunk(0)
                stage1_chunk(1)
                attn_window(0)
                stage1_chunk(2)
                stage1_chunk(3)
                attn_window(1)
    nc.finalize()
    return nc


def _get_nc(repeat: int = 1) -> bass.Bass:
    key = ("nc", repeat)
    if key not in _CACHE:
        _CACHE[key] = _build_bass(repeat)
    return _CACHE[key]


def _make_in_maps(x, w_attn, w_proj):
    x2 = np.ascontiguousarray(np.asarray(x, dtype=np.float32).reshape(T, C).T)  # [C, T]
    wr = np.asarray(w_attn, dtype=np.float32).reshape(C, 4, 6, D)
    wp = np.asarray(w_proj, dtype=np.float32)
    in_maps = []
    for c in range(8):
        g = c // 2
        s = (2 * c) % 4
        wqkv_c = np.ascontiguousarray(
            np.concatenate([wr[:, g, s, :], wr[:, g, s + 1, :], wr[:, g, 4, :], wr[:, g, 5, :]], axis=1)
        )
        wproj_c = np.ascontiguousarray(wp[128 * c : 128 * (c + 1), :])
        in_maps.append({"xt": x2, "wqkv": wqkv_c, "wproj": wproj_c})
    return in_maps


def _combine(results):
    acc = np.zeros((T, C), np.float64)
    for r in results:
        acc += r["out"]
    return acc.astype(np.float32).reshape(1, T, C)


def run_for_test(inputs, trace=False):
    """Returns (output, exec_time_ns_or_None). Used by test.py."""
    nc = _get_nc()
    in_maps = _make_in_maps(**inputs)
    res = run_bass_kernel_spmd(nc, in_maps, core_ids=list(range(8)), trace=trace)
    return _combine(res.results), res.exec_time_ns


def kernel(x, w_attn, w_proj):
    out, _ = run_for_test({"x": x, "w_attn": w_attn, "w_proj": w_proj})
    return out



# revision 27
# speedup vs baseline: 1.2287x; 1.2287x over previous
"""Causal GQA self-attention (B=1, T=2048, C=1024, 16 q-heads, 4 kv-groups, d=64)
on 8 Trainium2 NeuronCores.

Sharding: tensor-parallel over heads. Core c owns q-heads (2c, 2c+1) and kv-group
c//2. Each core computes x @ w_attn for its slice (transposed layout), RoPE,
causal flash-style attention for its 2 heads, and its partial y @ w_proj
(contracting only its 128 head-dims). Host sums the 8 partial outputs.

Pipeline (per core): x^T streams in as 4 column-chunks of 512; each chunk's
QKV projection + RoPE (stage1) feeds an attention window of 512 queries
(window w needs k-tiles 0..4w+3 only), whose epilogue projects and stores
directly from PSUM. Layout notes:
  - scores computed transposed sT[k, q]; softmax denominator via a ones
    column in the v operand; no max-subtraction (|scores| <= ~7).
  - vaug free layout [ones | v | ones] (66 cols): h0 uses cols 1:66 -> out
    partitions 0:65 (den at 64); h1 uses cols 0:65 -> out partitions 63:128
    (den at 63). No cross-partition moves needed to assemble y.
  - pt (exp output) and vaug are bf16: halves att@v operand traffic and
    avoids the 4x fp32r penalty on <256-free matmuls; accumulation stays f32.
  - causal mask on the diagonal 128x128 block added via a bf16 identity
    matmul of a -8e29 mask tile.
  - normalization: reciprocal of den row (DVE), PE broadcast to 64 rows,
    scale on DVE (h0) / Pool (h1); proj accumulates in PSUM and stores
    straight from PSUM to DRAM.
"""

import numpy as np

import concourse.bass as bass
import concourse.mybir as mybir
from concourse import bacc
import concourse.tile as tile
from concourse.bass_utils import run_bass_kernel_spmd

T = 2048
C = 1024
D = 64
CW = 512                      # chunk / attention-window width
NW = T // CW                  # 4 windows
F32 = mybir.dt.float32
F32R = mybir.dt.float32r
BF16 = mybir.dt.bfloat16
EXP = mybir.ActivationFunctionType.Exp

_CACHE: dict = {}


def _rope_tables():
    # Replicate reference.apply_rope's f32 pipeline exactly.
    inv = (1.0 / (np.float32(10000.0) ** (np.arange(0, D, 2, dtype=np.float32) / np.float32(D)))).astype(np.float32)
    freqs = (np.arange(T, dtype=np.float32)[:, None] * inv[None, :]).astype(np.float32)  # (T, 32)
    freqs = np.repeat(freqs, 2, axis=1)                                                  # (T, 64)
    cos = np.cos(freqs).astype(np.float32).T.copy()                                      # (64, T)
    sin = np.sin(freqs).astype(np.float32).T.copy()
    cos2 = np.ascontiguousarray(np.concatenate([cos, cos], axis=0))                      # (128, T)
    sin2 = np.ascontiguousarray(np.concatenate([sin, sin], axis=0))
    return cos2, sin2


def _const_mats():
    # perm (as lhsT): rot[2i] = -x[2i+1], rot[2i+1] = +x[2i]
    perm = np.zeros((128, 128), np.float32)
    for i in range(64):
        perm[2 * i + 1, 2 * i] = -1.0
        perm[2 * i, 2 * i + 1] = 1.0
    ident = np.eye(128, dtype=np.float32)
    shift = np.zeros((128, 128), np.float32)   # [64+i, i] = 1 down-shift; [i, 64+i] = 1 up-shift
    for i in range(64):
        shift[64 + i, i] = 1.0
        shift[i, 64 + i] = 1.0
    kq = np.arange(128)
    binmask = np.where(kq[:, None] <= kq[None, :], 0.0, -8e29).astype(np.float32)  # [k, q] additive
    return perm, ident, shift, binmask


def _build_bass(repeat: int = 1) -> bass.Bass:
    nc = bacc.Bacc(None, target_bir_lowering=False)
    xt_d = nc.dram_tensor("xt", [C, T], F32R, kind="ExternalInput")
    wqkv = nc.dram_tensor("wqkv", [C, 4 * D], F32R, kind="ExternalInput")
    wproj = nc.dram_tensor("wproj", [2 * D, C], F32R, kind="ExternalInput")
    out = nc.dram_tensor("out", [T, C], BF16, kind="ExternalOutput")

    cos2_np, sin2_np = _rope_tables()
    perm_np, ident_np, shift_np, binmask_np = _const_mats()
    cmat_np = np.ascontiguousarray(
        np.concatenate([perm_np, ident_np, shift_np, binmask_np], axis=1)
    )  # [128, 512]: perm | ident | shift | binmask
    cos_d = nc.inline_tensor(cos2_np, name="cos2")
    sin_d = nc.inline_tensor(sin2_np, name="sin2")
    cmat_d = nc.inline_tensor(cmat_np, name="cmatm")

    with tile.TileContext(nc) as tc:
        with (
            nc.allow_low_precision(reason="fp32r/bf16 rounding of matmul operands"),
            tc.tile_pool(name="const", bufs=1) as const,
            tc.tile_pool(name="big", bufs=1) as big,
            tc.tile_pool(name="work", bufs=1) as work,
            tc.tile_pool(name="ps", bufs=1, space="PSUM") as psp,
        ):
            SB = 3  # shared 1-bank psum slots

            # Everything rides the SP HWDGE queue in dependency-need order:
            # weights, chunk 0, constants, then later chunks interleaved with
            # their cos/sin slices (transfer order == issue order, so early
            # consumers aren't stuck behind 8MB of x^T).
            w_r = const.tile([128, 8, 4 * D], F32R)
            wqkv_src = wqkv.rearrange("(c p) n -> p c n", p=128)
            nc.sync.dma_start(out=w_r[:, :, 0:128], in_=wqkv_src[:, :, 0:128])
            xt = big.tile([128, 8, T], F32R)  # x^T as 8 c-tiles (host-transposed input)
            xt_src = xt_d.rearrange("(c p) t -> p c t", p=128)

            def load_xt_chunk(w):
                sl = slice(CW * w, CW * (w + 1))
                nc.sync.dma_start(out=xt[:, :, sl], in_=xt_src[:, :, sl])

            load_xt_chunk(0)
            nc.sync.dma_start(out=w_r[:, :, 128:256], in_=wqkv_src[:, :, 128:256])

            cmat_f = const.tile([128, 512], F32)
            nc.sync.dma_start(out=cmat_f, in_=cmat_d[:, :])
            id_sb = cmat_f[:, 128:256]
            cos_sb = const.tile([128, T], F32)
            sin_sb = const.tile([128, T], F32)

            def load_trig_chunk(w):
                sl = slice(CW * w, CW * (w + 1))
                nc.sync.dma_start(out=cos_sb[:, sl], in_=cos_d[:, sl])
                nc.sync.dma_start(out=sin_sb[:, sl], in_=sin_d[:, sl])

            load_trig_chunk(0)
            # w_proj halves, both at partitions 0:64 so per-head proj matmuls
            # avoid cross-partition moves of y
            wpc0 = const.tile([64, C], F32R)
            nc.sync.dma_start(out=wpc0, in_=wproj[0:64, :])
            wpc1 = const.tile([64, C], F32R)
            nc.sync.dma_start(out=wpc1, in_=wproj[64:128, :])

            perm_sb = const.tile([128, 128], F32R)
            nc.vector.tensor_copy(out=perm_sb, in_=cmat_f[:, 0:128])
            sh_sb = const.tile([128, 128], F32R)
            nc.vector.tensor_copy(out=sh_sb, in_=cmat_f[:, 256:384])
            idb_sb = const.tile([128, 128], BF16)
            nc.vector.tensor_copy(out=idb_sb, in_=cmat_f[:, 128:256])
            mkb_sb = const.tile([128, 128], BF16)
            nc.vector.tensor_copy(out=mkb_sb, in_=cmat_f[:, 384:512])
            ones_f = const.tile([128, 64], F32)
            nc.vector.memset(ones_f, 1.0)
            ones_sb = const.tile([65, 64], F32R)
            nc.vector.tensor_copy(out=ones_sb, in_=ones_f[0:65, :])

            for _w in range(1, NW):
                load_xt_chunk(_w)
                load_trig_chunk(_w)

            # persistent per-core activations
            qrope_sb = big.tile([128, T], F32R)   # roped q, h0 rows 0:64, h1 rows 64:128
            q1_sb = big.tile([64, T], F32R)       # roped q of h1 shifted to partitions 0:64
            krope_sb = big.tile([64, T], F32R)
            # v tiles in [t, d] layout, bf16, free cols: [v(64) | ones]
            vaug_sb = big.tile([128, 16, 65], BF16)
            nc.vector.memset(vaug_sb[:, :, 64:65], 1.0)

            # ------- stage 1: QKV projection + RoPE for one 512-chunk of T.
            # Generator: yields between pieces so the driver can weave these
            # instructions between the previous window's attention steps.
            def stage1_units(r, w):
                sl = slice(CW * w, CW * (w + 1))
                qraw = work.tile([128, CW], F32R, tag="qraw", bufs=2, name=f"x{r}qraw{w}")
                kvraw = work.tile([128, CW], F32R, tag="kvraw", bufs=2, name=f"x{r}kvraw{w}")
                for m, dst in ((0, qraw), (1, kvraw)):
                    ps = psp.tile([128, CW], F32, tag="s", bufs=SB, name=f"x{r}qkv{w}_{m}")
                    for c in range(8):
                        nc.tensor.matmul(
                            ps,
                            lhsT=w_r[:, c, 128 * m : 128 * (m + 1)],
                            rhs=xt[:, c, sl],
                            start=(c == 0),
                            stop=(c == 7),
                        )
                        if c == 3:
                            yield
                    nc.vector.tensor_copy(out=dst, in_=ps)
                    yield
                tmp = work.tile([128, CW], F32, tag="tmp", bufs=2, name=f"x{r}tmp{w}")
                tmpk = work.tile([64, CW], F32, tag="tmpk", bufs=2, name=f"x{r}tmpk{w}")
                # rope q (both heads at once)
                rps = psp.tile([128, CW], F32, tag="s", bufs=SB, name=f"x{r}rot{w}")
                nc.tensor.matmul(rps, lhsT=perm_sb, rhs=qraw, start=True, stop=True)
                nc.vector.tensor_mul(tmp, rps, sin_sb[:, sl])
                nc.gpsimd.tensor_mul(qrope_sb[:, sl], qraw, cos_sb[:, sl])
                nc.gpsimd.tensor_add(qrope_sb[:, sl], qrope_sb[:, sl], tmp)
                yield
                # rope k (rows 0:64 of kvraw)
                rpsk = psp.tile([64, CW], F32, tag="s", bufs=SB, name=f"x{r}rotk{w}")
                nc.tensor.matmul(rpsk, lhsT=perm_sb[0:64, 0:64], rhs=kvraw[0:64, :], start=True, stop=True)
                nc.vector.tensor_mul(tmpk, rpsk, sin_sb[0:64, sl])
                nc.gpsimd.tensor_mul(krope_sb[:, sl], kvraw[0:64, :], cos_sb[0:64, sl])
                nc.gpsimd.tensor_add(krope_sb[:, sl], krope_sb[:, sl], tmpk)
                yield
                # shift roped h1 q down to partitions 0:64
                sps = psp.tile([64, CW], F32, tag="s", bufs=SB, name=f"x{r}shift{w}")
                nc.tensor.matmul(sps, lhsT=sh_sb[64:128, 0:64], rhs=qrope_sb[64:128, sl], start=True, stop=True)
                nc.vector.tensor_copy(out=q1_sb[:, sl], in_=sps)
                yield
                # v_aug: transpose v tiles of this chunk into [t, d] bf16
                for tt in range(4 * w, 4 * w + 4):
                    vps = psp.tile([128, 64], F32, tag="s", bufs=SB, name=f"x{r}vtr{tt}")
                    nc.tensor.transpose(
                        vps,
                        in_=kvraw[64:128, 128 * (tt - 4 * w) : 128 * (tt - 4 * w + 1)].bitcast(F32),
                        identity=id_sb[64:128, 64:128],
                    )
                    nc.vector.tensor_copy(out=vaug_sb[:, tt, 0:64], in_=vps)
                    if tt % 2 == 1:
                        yield

            # ------- attention window of CW queries (both heads); yields per
            # k-tile step so stage1(w+1) / epilogue(w-1) can interleave.
            def attn_units(r, w, out_yps):
                ktiles = 4 * w + 4
                yps = {
                    h: psp.tile([65, CW], F32, tag=f"yt{h}", bufs=2, name=f"x{r}yps{w}_{h}")
                    for h in range(2)
                }
                out_yps.update(yps)

                def emit_yt(jj, ptsj):
                    gg = jj - 4 * w
                    qq0 = max(gg, 0) * 128
                    for h in range(2):
                        nc.tensor.matmul(
                            yps[h][:, qq0:CW],
                            lhsT=vaug_sb[:, jj, :],
                            rhs=ptsj[h][:, 0 : CW - qq0],
                            start=(jj == 0),
                            stop=(jj == ktiles - 1),
                            skip_group_check=True,
                        )

                prev = None
                for j in range(ktiles):
                    g = j - 4 * w
                    q0 = max(g, 0) * 128
                    pts = {}
                    for h in range(2):
                        qsrc = qrope_sb if h == 0 else q1_sb
                        pt = work.tile([128, CW], BF16, tag="pt", bufs=6, name=f"x{r}pt{w}_{j}_{h}")
                        spsm = psp.tile([128, CW], F32, tag="s", bufs=SB, name=f"x{r}s{w}_{j}_{h}")
                        nc.tensor.matmul(
                            spsm[:, 0 : CW - q0],
                            lhsT=krope_sb[:, 128 * j : 128 * (j + 1)],
                            rhs=qsrc[0:64, CW * w + q0 : CW * (w + 1)],
                            start=True,
                            stop=(g < 0),
                            skip_group_check=True,
                        )
                        if g >= 0:
                            nc.tensor.matmul(
                                spsm[:, 0:128],
                                lhsT=idb_sb,
                                rhs=mkb_sb,
                                start=False,
                                stop=True,
                                skip_group_check=True,
                            )
                        nc.scalar.activation(
                            out=pt[:, 0 : CW - q0], in_=spsm[:, 0 : CW - q0], func=EXP, scale=0.125
                        )
                        pts[h] = pt
                    if prev is not None:
                        emit_yt(*prev)
                    prev = (j, pts)
                    yield
                emit_yt(*prev)

            # ------- epilogue: normalize, project, store; yields between
            # pieces so it can hide under the next window's attention.
            def epilogue_units(r, w, yps):
                r0 = work.tile([65, CW], F32R, tag="r0", bufs=2, name=f"x{r}r0_{w}")
                r1 = work.tile([65, CW], F32R, tag="r1", bufs=2, name=f"x{r}r1_{w}")
                nc.vector.reciprocal(out=r0[64:65, :], in_=yps[0][64:65, :])
                nc.vector.reciprocal(out=r1[64:65, :], in_=yps[1][64:65, :])
                rbps0 = psp.tile([64, CW], F32, tag="s", bufs=SB, name=f"x{r}rb0_{w}")
                nc.tensor.matmul(
                    rbps0, lhsT=ones_sb[64:65, 0:64], rhs=r0[64:65, :],
                    start=True, stop=True, skip_group_check=True,
                )
                rbps1 = psp.tile([64, CW], F32, tag="s", bufs=SB, name=f"x{r}rb1_{w}")
                nc.tensor.matmul(
                    rbps1, lhsT=ones_sb[64:65, 0:64], rhs=r1[64:65, :],
                    start=True, stop=True, skip_group_check=True,
                )
                rb0_sb = work.tile([64, CW], F32, tag="rb0", bufs=2, name=f"x{r}rbs0_{w}")
                nc.vector.tensor_copy(out=rb0_sb, in_=rbps0)
                rb1_sb = work.tile([64, CW], F32, tag="rb1", bufs=2, name=f"x{r}rbs1_{w}")
                nc.vector.tensor_copy(out=rb1_sb, in_=rbps1)
                yield
                yn0 = work.tile([64, CW], F32R, tag="yn0", bufs=2, name=f"x{r}yn0_{w}")
                yn1 = work.tile([64, CW], F32R, tag="yn1", bufs=2, name=f"x{r}yn1_{w}")
                nc.vector.tensor_mul(yn0, yps[0][0:64, :], rb0_sb)
                nc.vector.tensor_mul(yn1, yps[1][0:64, :], rb1_sb)
                yield
                for t4 in range(4):
                    tglob = 4 * w + t4
                    tsl = slice(128 * t4, 128 * (t4 + 1))
                    osb = work.tile([128, C], BF16, tag="o", bufs=3, name=f"x{r}o{w}_{t4}")
                    for n2 in range(2):
                        nsl = slice(512 * n2, 512 * (n2 + 1))
                        ops_ = psp.tile([128, 512], F32, tag="o", bufs=1, name=f"x{r}op{w}_{t4}_{n2}")
                        nc.tensor.matmul(
                            ops_, lhsT=yn0[:, tsl], rhs=wpc0[:, nsl],
                            start=True, stop=False, skip_group_check=True,
                        )
                        nc.tensor.matmul(
                            ops_, lhsT=yn1[:, tsl], rhs=wpc1[:, nsl],
                            start=False, stop=True, skip_group_check=True,
                        )
                        nc.vector.tensor_copy(out=osb[:, nsl], in_=ops_)
                    nc.sync.dma_start(out=out[128 * tglob : 128 * (tglob + 1), :], in_=osb)
                    yield

            def drain(gen, n=None):
                if gen is None:
                    return None
                try:
                    if n is None:
                        for _ in gen:
                            pass
                        return None
                    for _ in range(n):
                        next(gen)
                    return gen
                except StopIteration:
                    return None

            # ------- software-pipelined drive over (rep, window) -------
            first = stage1_units(0, 0)
            drain(first)
            epi_pend = None
            for r in range(repeat):
                for w in range(NW):
                    if w == NW - 1 and r + 1 < repeat:
                        # next rep's x^T reloads; transfers overlap this window
                        for cw in range(NW):
                            load_xt_chunk(cw)
                    if w + 1 < NW:
                        nxt = stage1_units(r, w + 1)
                    elif r + 1 < repeat:
                        nxt = stage1_units(r + 1, 0)
                    else:
                        nxt = None
                    yps = {}
                    for _ in attn_units(r, w, yps):
                        nxt = drain(nxt, 3)
                        epi_pend = drain(epi_pend, 1)
                    drain(nxt)
                    drain(epi_pend)
                    epi_pend = epilogue_units(r, w, yps)
            drain(epi_pend)
    nc.finalize()
    return nc


def _get_nc(repeat: int = 1) -> bass.Bass:
    key = ("nc", repeat)
    if key not in _CACHE:
        _CACHE[key] = _build_bass(repeat)
    return _CACHE[key]


def _make_in_maps(x, w_attn, w_proj):
    x2 = np.ascontiguousarray(np.asarray(x, dtype=np.float32).reshape(T, C).T)  # [C, T]
    wr = np.asarray(w_attn, dtype=np.float32).reshape(C, 4, 6, D)
    wp = np.asarray(w_proj, dtype=np.float32)
    in_maps = []
    for c in range(8):
        g = c // 2
        s = (2 * c) % 4
        wqkv_c = np.ascontiguousarray(
            np.concatenate([wr[:, g, s, :], wr[:, g, s + 1, :], wr[:, g, 4, :], wr[:, g, 5, :]], axis=1)
        )
        wproj_c = np.ascontiguousarray(wp[128 * c : 128 * (c + 1), :])
        in_maps.append({"xt": x2, "wqkv": wqkv_c, "wproj": wproj_c})
    return in_maps


def _combine(results):
    acc = np.zeros((T, C), np.float64)
    for r in results:
        acc += r["out"]
    return acc.astype(np.float32).reshape(1, T, C)


def run_for_test(inputs, trace=False):
    """Returns (output, exec_time_ns_or_None). Used by test.py."""
    nc = _get_nc()
    in_maps = _make_in_maps(**inputs)
    res = run_bass_kernel_spmd(nc, in_maps, core_ids=list(range(8)), trace=trace)
    return _combine(res.results), res.exec_time_ns


def kernel(x, w_attn, w_proj):
    out, _ = run_for_test({"x": x, "w_attn": w_attn, "w_proj": w_proj})
    return out
